# revision 21
# baseline (speedup 1.0000x reference)
"""Trainium2 Bass kernel for BertSelfAttentionWithRelations (RAT-style).

Sharding: 8 cores = 4 batches x 2 query-row halves; each core runs full
12-head attention for its (batch, 512 query rows) slab.

Factorized relation handling: softmax(qk/8 + qrel[i,rel]/8 + mask) is
computed as E = exp((qk+8*mask)/8) scaled per relation bin by g = exp(qrel/8):
  u_r = (E (.) M_r)^T @ [v | 1] PV matmuls give both ctx partials and bin
  sums c_r via the ones column, so there are no per-head elementwise
  mask-reduce passes over S x S.
  ctx = [g_0*u_tot + sum_r (g_r-g_0)*u_r + pr@rel_v] / Z,  pr_r = g_r*c_r,
  Z = sum_r pr_r (falls out of the same accumulations).
"""

from contextlib import ExitStack

import numpy as np

import concourse.bass as bass
import concourse.mybir as mybir
import concourse.tile as tile
from concourse import bacc
from concourse.bass_utils import run_bass_kernel_spmd
from concourse.masks import make_identity

F32 = mybir.dt.float32
F32R = mybir.dt.float32r
BF16 = mybir.dt.bfloat16
AF = mybir.ActivationFunctionType
ALU = mybir.AluOpType

B, S, HS, H, D = 4, 1024, 768, 12, 64
SH = S // 2          # rows per core
NIT = SH // 128      # 4 i-tiles per core
NC_CH = HS // 128    # 6 channel chunks
NJT = S // 128       # 8 j-chunks
VW = 65              # v block width per head (64 + ones column)


def _build_nc():
    nc = bacc.Bacc("TRN2", target_bir_lowering=False, debug=False, num_devices=8)

    dt_in = dict(kind="ExternalInput")
    hT = nc.dram_tensor("hT", [128, NC_CH, S], F32R, **dt_in).ap()
    hTq = nc.dram_tensor("hTq", [128, NC_CH, SH], F32R, **dt_in).ap()
    wq = nc.dram_tensor("wq", [128, NC_CH, HS], F32R, **dt_in).ap()
    wk = nc.dram_tensor("wk", [128, NC_CH, HS], F32R, **dt_in).ap()
    wv = nc.dram_tensor("wv", [128, NC_CH, HS], F32R, **dt_in).ap()
    bqc = nc.dram_tensor("bqc", [128, NC_CH], F32, **dt_in).ap()
    bkc = nc.dram_tensor("bkc", [128, NC_CH], F32, **dt_in).ap()
    bvrow = nc.dram_tensor("bvrow", [1, HS], F32R, **dt_in).ap()
    relTf = nc.dram_tensor("relTf", [128, NJT, SH], F32, **dt_in).ap()
    mask8 = nc.dram_tensor("mask8", [1, S], F32R, **dt_in).ap()
    rkT = nc.dram_tensor("rkT", [128, 8], F32R, **dt_in).ap()
    rv = nc.dram_tensor("rv", [7, D], F32R, **dt_in).ap()
    out = nc.dram_tensor("out", [128, NIT, HS], F32, kind="ExternalOutput").ap()

    with tile.TileContext(nc) as tc, ExitStack() as ctx:
        # ---- persistent pools -------------------------------------------
        persist = ctx.enter_context(tc.tile_pool(name="persist", bufs=1))
        qTs = persist.tile([128, NC_CH, SH], F32R, tag="qTs")
        kTs = persist.tile([128, NC_CH, S], F32R, tag="kTs")
        vs = persist.tile([128, NJT, VW * H], BF16, tag="vs")
        relT_sb = persist.tile([128, NJT, SH], F32, tag="relT")
        out_sb = persist.tile([128, NIT, HS], F32, tag="outsb")
        mask8_sb = persist.tile([1, S], F32R, tag="mask8")
        rkT_sb = persist.tile([128, 8], F32R, tag="rkT")
        rv_sb = persist.tile([7, D], F32R, tag="rv")
        ident16 = persist.tile([128, 128], BF16, tag="ident16")
        identf = persist.tile([128, 128], F32, tag="identf")
        ones1 = persist.tile([1, 256], F32R, tag="ones1")
        ones1f = persist.tile([1, 256], F32, tag="ones1f")

        nc.sync.dma_start(relT_sb[:], relTf[:])
        nc.sync.dma_start(mask8_sb[:], mask8[:])
        nc.sync.dma_start(rkT_sb[:], rkT[:])
        nc.sync.dma_start(rv_sb[:], rv[:])
        make_identity(nc, identf[:])
        nc.vector.tensor_copy(ident16[:], identf[:])
        nc.gpsimd.memset(ones1f[:], 1.0)
        nc.vector.tensor_copy(ones1[:], ones1f[:])
        nc.gpsimd.memset(vs[:, :, 64::VW], 1.0)  # ones columns

        # ---- stage A: projections ---------------------------------------
        with tc.tile_pool(name="stageA", bufs=1) as sa, \
             tc.tile_pool(name="projpsum", bufs=3, space="PSUM") as pps:
            hT_sb = sa.tile([128, NC_CH, S], F32R, tag="hT")
            hTq_sb = sa.tile([128, NC_CH, SH], F32R, tag="hTq")
            wq_sb = sa.tile([128, NC_CH, HS], F32R, tag="wq")
            wk_sb = sa.tile([128, NC_CH, HS], F32R, tag="wk")
            wv_sb = sa.tile([128, NC_CH, HS], F32R, tag="wv")
            bq_sb = sa.tile([128, NC_CH], F32, tag="bq")
            bk_sb = sa.tile([128, NC_CH], F32, tag="bk")
            bv_sb = sa.tile([1, HS], F32R, tag="bv")
            nc.sync.dma_start(hT_sb[:], hT[:])
            nc.sync.dma_start(hTq_sb[:], hTq[:])
            nc.sync.dma_start(wq_sb[:], wq[:])
            nc.sync.dma_start(wk_sb[:], wk[:])
            nc.sync.dma_start(wv_sb[:], wv[:])
            nc.sync.dma_start(bq_sb[:], bqc[:])
            nc.sync.dma_start(bk_sb[:], bkc[:])
            nc.sync.dma_start(bv_sb[:], bvrow[:])

            # qT / kT in transposed [hd, i] layout
            for m in range(NC_CH):
                ps = pps.tile([128, 512], F32, tag="pp")
                for n in range(NC_CH):
                    nc.tensor.matmul(
                        ps[:], wq_sb[:, n, m * 128:(m + 1) * 128], hTq_sb[:, n, :],
                        start=(n == 0), stop=(n == NC_CH - 1),
                    )
                nc.vector.tensor_scalar_add(qTs[:, m, :], ps[:], bq_sb[:, m:m + 1])
            for m in range(NC_CH):
                for jh in range(2):
                    ps = pps.tile([128, 512], F32, tag="pp")
                    for n in range(NC_CH):
                        nc.tensor.matmul(
                            ps[:], wk_sb[:, n, m * 128:(m + 1) * 128],
                            hT_sb[:, n, jh * 512:(jh + 1) * 512],
                            start=(n == 0), stop=(n == NC_CH - 1),
                        )
                    nc.vector.tensor_scalar_add(
                        kTs[:, m, jh * 512:(jh + 1) * 512], ps[:], bk_sb[:, m:m + 1]
                    )

            # v natural [j, hd] + bias, evicted per head into 65-wide blocks
            for jt in range(NJT):
                for half in range(2):
                    ps = pps.tile([128, 384], F32, tag="ppv")
                    for n in range(NC_CH):
                        nc.tensor.matmul(
                            ps[:], hT_sb[:, n, jt * 128:(jt + 1) * 128],
                            wv_sb[:, n, half * 384:(half + 1) * 384],
                            start=(n == 0), stop=False,
                        )
                    nc.tensor.matmul(
                        ps[:], ones1[:, 0:128], bv_sb[:, half * 384:(half + 1) * 384],
                        start=False, stop=True,
                    )
                    for hh in range(6):
                        h = half * 6 + hh
                        nc.scalar.copy(
                            vs[:, jt, h * VW:h * VW + 64], ps[:, hh * 64:(hh + 1) * 64]
                        )

        # ---- stage B/C: attention ---------------------------------------
        # i-tiles processed in pairs (256 query cols) so the transposed score
        # matmuls and mask-multiplies run at full fp32r/bf16 rate.
        mpool = ctx.enter_context(tc.tile_pool(name="masks", bufs=2))
        epool = ctx.enter_context(tc.tile_pool(name="ework", bufs=2))
        spool = ctx.enter_context(tc.tile_pool(name="small", bufs=6))
        scps = ctx.enter_context(tc.tile_pool(name="scps", bufs=2, space="PSUM"))
        ups = ctx.enter_context(tc.tile_pool(name="ups", bufs=2, space="PSUM"))
        yps = ctx.enter_context(tc.tile_pool(name="yps", bufs=1, space="PSUM"))

        for itp in range(NIT // 2):
            # transposed-relation onehot masks [j-part, r, jc, i(256)] bf16
            mskT = mpool.tile([128, 6, NJT, 256], BF16, tag="mskT")
            for r in range(1, 7):
                nc.vector.tensor_scalar(
                    mskT[:, r - 1, :, :], relT_sb[:, :, itp * 256:(itp + 1) * 256],
                    float(r), None, ALU.is_equal,
                )

            for h in range(H):
                po = (h % 2) * 64
                mch = h // 2
                qT_p = qTs[po:po + 64, mch, itp * 256:(itp + 1) * 256]

                # transposed scores scT[j, i] = k.q + mask[j], exp -> ET bf16
                ET = epool.tile([128, NJT, 256], BF16, tag="ET")
                for half in range(2):
                    scp = scps.tile([128, 4, 256], F32, tag="sc")
                    for c in range(4):
                        jc = half * 4 + c
                        nc.tensor.matmul(
                            scp[:, c, :],
                            kTs[po:po + 64, mch, jc * 128:(jc + 1) * 128],
                            qT_p, start=True, stop=False,
                        )
                        nc.tensor.matmul(
                            scp[:, c, :],
                            mask8_sb[0:1, jc * 128:(jc + 1) * 128],
                            ones1[:], start=False, stop=True,
                        )
                    nc.scalar.activation(
                        ET[:, half * 4:(half + 1) * 4, :], scp[:], AF.Exp, scale=0.125
                    )

                # per-bin masked copies of ET (pair-wide)
                ErTs = []
                for r in range(1, 7):
                    ErT = epool.tile([128, NJT, 256], BF16, tag=f"ErT{r}")
                    eng = nc.vector if r <= 4 else nc.gpsimd
                    eng.tensor_tensor(ErT[:], ET[:], mskT[:, r - 1, :, :], ALU.mult)
                    ErTs.append(ErT)

                for a in range(2):
                    it = itp * 2 + a
                    isl = slice(a * 128, (a + 1) * 128)
                    qT_h = qTs[po:po + 64, mch, it * 128:(it + 1) * 128]

                    # g = exp(qrel/8) and dg = g_r - g_0
                    qrel = yps.tile([128, 8], F32, tag="y")
                    nc.tensor.matmul(
                        qrel[:], qT_h, rkT_sb[po:po + 64, :], start=True, stop=True,
                    )
                    g = spool.tile([128, 8], F32, tag="g")
                    nc.scalar.activation(g[:, 0:7], qrel[:, 0:7], AF.Exp, scale=0.125)
                    dg = spool.tile([128, 6], F32, tag="dg")
                    nc.vector.tensor_scalar(
                        dg[:], g[:, 1:7], g[:, 0:1], None, ALU.subtract
                    )

                    acc = spool.tile([128, VW], F32, tag="acc")
                    pr = spool.tile([128, 8], F32, tag="pr")
                    u = ups.tile([128, VW], F32, tag="u")
                    for jc in range(NJT):
                        nc.tensor.matmul(
                            u[:], ET[:, jc, isl], vs[:, jc, h * VW:(h + 1) * VW],
                            start=(jc == 0), stop=(jc == NJT - 1),
                        )
                    nc.vector.tensor_scalar(acc[:], u[:], g[:, 0:1], None, ALU.mult)
                    for r in range(1, 7):
                        u = ups.tile([128, VW], F32, tag="u")
                        for jc in range(NJT):
                            nc.tensor.matmul(
                                u[:], ErTs[r - 1][:, jc, isl],
                                vs[:, jc, h * VW:(h + 1) * VW],
                                start=(jc == 0), stop=(jc == NJT - 1),
                            )
                        nc.vector.scalar_tensor_tensor(
                            acc[:], u[:], dg[:, r - 1:r], acc[:],
                            op0=ALU.mult, op1=ALU.add,
                        )
                        nc.vector.tensor_scalar(
                            pr[:, r:r + 1], u[:, 64:65], g[:, r:r + 1], None, ALU.mult
                        )

                    # pr_0 = Z - sum pr_r ; rel-v term; normalize + emit
                    prs = spool.tile([128, 1], F32, tag="prs")
                    nc.vector.tensor_reduce(
                        prs[:], pr[:, 1:7], mybir.AxisListType.X, ALU.add
                    )
                    nc.vector.tensor_scalar(
                        pr[:, 0:1], acc[:, 64:65], prs[:, 0:1], None, ALU.subtract
                    )
                    prT = yps.tile([8, 128], F32, tag="y2")
                    nc.tensor.transpose(prT[0:7, :], pr[:, 0:7], identf[:])
                    prT_sb = spool.tile([8, 128], F32R, tag="prTs")
                    nc.scalar.copy(prT_sb[0:7, :], prT[0:7, :])
                    cxr = yps.tile([128, 64], F32, tag="y2")
                    nc.tensor.matmul(
                        cxr[:], prT_sb[0:7, :], rv_sb[:], start=True, stop=False,
                    )
                    nc.tensor.matmul(
                        cxr[:], identf[:], acc[:, 0:64],
                        start=False, stop=True, skip_group_check=True,
                    )
                    rz = spool.tile([128, 1], F32, tag="rz")
                    nc.vector.reciprocal(rz[:], acc[:, 64:65])
                    nc.scalar.activation(
                        out_sb[:, it, h * 64:(h + 1) * 64], cxr[:], AF.Copy,
                        scale=rz[:],
                    )

            for a in range(2):
                it = itp * 2 + a
                nc.sync.dma_start(out[:, it, :], out_sb[:, it, :])

    nc.compile()
    return nc


_NC_CACHE = []


def _get_nc():
    if not _NC_CACHE:
        _NC_CACHE.append(_build_nc())
    return _NC_CACHE[0]


def _marshal(hidden_states, attention_mask, relation, Wq, bq, Wk, bk, Wv, bv,
             rel_k_emb, rel_v_emb):
    f32 = np.float32
    hidden_states = np.asarray(hidden_states, f32)
    attention_mask = np.asarray(attention_mask, f32)
    relation = np.asarray(relation)
    Wq, Wk, Wv = (np.ascontiguousarray(np.asarray(w, f32)) for w in (Wq, Wk, Wv))
    bq, bk, bv = (np.asarray(x, f32) for x in (bq, bk, bv))

    def wchunk(w):
        return np.ascontiguousarray(w.reshape(NC_CH, 128, HS).transpose(1, 0, 2))

    shared = {
        "wq": wchunk(Wq), "wk": wchunk(Wk), "wv": wchunk(Wv),
        "bqc": np.ascontiguousarray(bq.reshape(NC_CH, 128).T),
        "bkc": np.ascontiguousarray(bk.reshape(NC_CH, 128).T),
        "bvrow": np.ascontiguousarray(bv.reshape(1, HS)),
        "rkT": np.ascontiguousarray(
            np.pad(np.tile(np.asarray(rel_k_emb, f32).T, (2, 1)), ((0, 0), (0, 1)))),
        "rv": np.ascontiguousarray(np.asarray(rel_v_emb, f32)),
    }
    in_maps = []
    for core in range(8):
        b, ih = core // 2, core % 2
        i0 = ih * SH
        hTm = np.ascontiguousarray(hidden_states[b].T)  # [HS, S]
        m = dict(shared)
        m["hT"] = np.ascontiguousarray(hTm.reshape(NC_CH, 128, S).transpose(1, 0, 2))
        m["hTq"] = np.ascontiguousarray(
            hTm[:, i0:i0 + SH].reshape(NC_CH, 128, SH).transpose(1, 0, 2))
        m["relTf"] = np.ascontiguousarray(
            relation[b, i0:i0 + SH].astype(f32).T.reshape(NJT, 128, SH).transpose(1, 0, 2))
        m["mask8"] = np.ascontiguousarray(
            (attention_mask[b, 0, 0] * 8.0).reshape(1, S).astype(f32))
        in_maps.append(m)
    return in_maps


def kernel(hidden_states, attention_mask, relation, Wq, bq, Wk, bk, Wv, bv,
           rel_k_emb, rel_v_emb, _trace=False, _tmpdir=None):
    nc = _get_nc()
    in_maps = _marshal(hidden_states, attention_mask, relation, Wq, bq, Wk, bk,
                       Wv, bv, rel_k_emb, rel_v_emb)
    kw = {}
    if _trace:
        kw = dict(trace=True, tmpdir=_tmpdir)
    res = run_bass_kernel_spmd(nc, in_maps, core_ids=list(range(8)), **kw)
    out = np.zeros((B, S, HS), np.float32)
    for core in range(8):
        b, ih = core // 2, core % 2
        o = res.results[core]["out"]  # [128, NIT, HS]
        out[b, ih * SH:(ih + 1) * SH] = o.transpose(1, 0, 2).reshape(SH, HS)
    if _trace:
        return out, res
    return out


# revision 24
# speedup vs baseline: 1.0001x; 1.0001x over previous
"""Trainium2 Bass kernel for BertSelfAttentionWithRelations (RAT-style).

Sharding: 8 cores = 4 batches x 2 query-row halves; each core runs full
12-head attention for its (batch, 512 query rows) slab.

Factorized relation handling: softmax(qk/8 + qrel[i,rel]/8 + mask) is
computed as E = exp((qk+8*mask)/8) scaled per relation bin by g = exp(qrel/8):
  u_r = (E (.) M_r)^T @ [v | 1] PV matmuls give both ctx partials and bin
  sums c_r via the ones column, so there are no per-head elementwise
  mask-reduce passes over S x S.
  ctx = [g_0*u_tot + sum_r (g_r-g_0)*u_r + pr@rel_v] / Z,  pr_r = g_r*c_r,
  Z = sum_r pr_r (falls out of the same accumulations).
"""

from contextlib import ExitStack

import numpy as np

import concourse.bass as bass
import concourse.mybir as mybir
import concourse.tile as tile
from concourse import bacc
from concourse.bass_utils import run_bass_kernel_spmd
from concourse.masks import make_identity

F32 = mybir.dt.float32
F32R = mybir.dt.float32r
BF16 = mybir.dt.bfloat16
AF = mybir.ActivationFunctionType
ALU = mybir.AluOpType

B, S, HS, H, D = 4, 1024, 768, 12, 64
SH = S // 2          # rows per core
NIT = SH // 128      # 4 i-tiles per core
NC_CH = HS // 128    # 6 channel chunks
NJT = S // 128       # 8 j-chunks
VW = 65              # v block width per head (64 + ones column)


def _build_nc():
    nc = bacc.Bacc("TRN2", target_bir_lowering=False, debug=False, num_devices=8)

    dt_in = dict(kind="ExternalInput")
    hT = nc.dram_tensor("hT", [128, NC_CH, S], F32R, **dt_in).ap()
    hTq = nc.dram_tensor("hTq", [128, NC_CH, SH], F32R, **dt_in).ap()
    wq = nc.dram_tensor("wq", [128, NC_CH, HS], F32R, **dt_in).ap()
    wk = nc.dram_tensor("wk", [128, NC_CH, HS], F32R, **dt_in).ap()
    wv = nc.dram_tensor("wv", [128, NC_CH, HS], F32R, **dt_in).ap()
    bqc = nc.dram_tensor("bqc", [128, NC_CH], F32, **dt_in).ap()
    bkc = nc.dram_tensor("bkc", [128, NC_CH], F32, **dt_in).ap()
    bvrow = nc.dram_tensor("bvrow", [1, HS], F32R, **dt_in).ap()
    relTf = nc.dram_tensor("relTf", [128, NJT, SH], F32, **dt_in).ap()
    mask8 = nc.dram_tensor("mask8", [1, S], F32R, **dt_in).ap()
    rkT = nc.dram_tensor("rkT", [128, 8], F32R, **dt_in).ap()
    rv = nc.dram_tensor("rv", [7, D], F32R, **dt_in).ap()
    out = nc.dram_tensor("out", [128, NIT, HS], F32, kind="ExternalOutput").ap()

    with tile.TileContext(nc) as tc, ExitStack() as ctx:
        # ---- persistent pools -------------------------------------------
        persist = ctx.enter_context(tc.tile_pool(name="persist", bufs=1))
        qTs = persist.tile([128, NC_CH, SH], F32R, tag="qTs")
        kTs = persist.tile([128, NC_CH, S], F32R, tag="kTs")
        vs = persist.tile([128, NJT, VW * H], BF16, tag="vs")
        relT_sb = persist.tile([128, NJT, SH], F32, tag="relT")
        out_sb = persist.tile([128, NIT, HS], F32, tag="outsb")
        mask8_sb = persist.tile([1, S], F32R, tag="mask8")
        rkT_sb = persist.tile([128, 8], F32R, tag="rkT")
        rv_sb = persist.tile([7, D], F32R, tag="rv")
        ident16 = persist.tile([128, 128], BF16, tag="ident16")
        identf = persist.tile([128, 128], F32, tag="identf")
        ones1 = persist.tile([1, 256], F32R, tag="ones1")
        ones1f = persist.tile([1, 256], F32, tag="ones1f")

        nc.sync.dma_start(relT_sb[:], relTf[:])
        nc.sync.dma_start(mask8_sb[:], mask8[:])
        nc.sync.dma_start(rkT_sb[:], rkT[:])
        nc.sync.dma_start(rv_sb[:], rv[:])
        make_identity(nc, identf[:])
        nc.vector.tensor_copy(ident16[:], identf[:])
        nc.gpsimd.memset(ones1f[:], 1.0)
        nc.vector.tensor_copy(ones1[:], ones1f[:])
        nc.gpsimd.memset(vs[:, :, 64::VW], 1.0)  # ones columns

        # ---- stage A: projections ---------------------------------------
        with tc.tile_pool(name="stageA", bufs=1) as sa, \
             tc.tile_pool(name="projpsum", bufs=3, space="PSUM") as pps:
            hT_sb = sa.tile([128, NC_CH, S], F32R, tag="hT")
            hTq_sb = sa.tile([128, NC_CH, SH], F32R, tag="hTq")
            wq_sb = sa.tile([128, NC_CH, HS], F32R, tag="wq")
            wk_sb = sa.tile([128, NC_CH, HS], F32R, tag="wk")
            wv_sb = sa.tile([128, NC_CH, HS], F32R, tag="wv")
            bq_sb = sa.tile([128, NC_CH], F32, tag="bq")
            bk_sb = sa.tile([128, NC_CH], F32, tag="bk")
            bv_sb = sa.tile([1, HS], F32R, tag="bv")
            nc.sync.dma_start(hT_sb[:], hT[:])
            nc.sync.dma_start(hTq_sb[:], hTq[:])
            nc.sync.dma_start(wq_sb[:], wq[:])
            nc.sync.dma_start(wk_sb[:], wk[:])
            nc.sync.dma_start(wv_sb[:], wv[:])
            nc.sync.dma_start(bq_sb[:], bqc[:])
            nc.sync.dma_start(bk_sb[:], bkc[:])
            nc.sync.dma_start(bv_sb[:], bvrow[:])

            # qT / kT in transposed [hd, i] layout
            for m in range(NC_CH):
                ps = pps.tile([128, 512], F32, tag="pp")
                for n in range(NC_CH):
                    nc.tensor.matmul(
                        ps[:], wq_sb[:, n, m * 128:(m + 1) * 128], hTq_sb[:, n, :],
                        start=(n == 0), stop=(n == NC_CH - 1),
                    )
                nc.vector.tensor_scalar_add(qTs[:, m, :], ps[:], bq_sb[:, m:m + 1])
            for m in range(NC_CH):
                for jh in range(2):
                    ps = pps.tile([128, 512], F32, tag="pp")
                    for n in range(NC_CH):
                        nc.tensor.matmul(
                            ps[:], wk_sb[:, n, m * 128:(m + 1) * 128],
                            hT_sb[:, n, jh * 512:(jh + 1) * 512],
                            start=(n == 0), stop=(n == NC_CH - 1),
                        )
                    nc.vector.tensor_scalar_add(
                        kTs[:, m, jh * 512:(jh + 1) * 512], ps[:], bk_sb[:, m:m + 1]
                    )

            # v natural [j, hd] + bias, evicted per head into 65-wide blocks
            for jt in range(NJT):
                for half in range(2):
                    ps = pps.tile([128, 384], F32, tag="ppv")
                    for n in range(NC_CH):
                        nc.tensor.matmul(
                            ps[:], hT_sb[:, n, jt * 128:(jt + 1) * 128],
                            wv_sb[:, n, half * 384:(half + 1) * 384],
                            start=(n == 0), stop=False,
                        )
                    nc.tensor.matmul(
                        ps[:], ones1[:, 0:128], bv_sb[:, half * 384:(half + 1) * 384],
                        start=False, stop=True,
                    )
                    for hh in range(6):
                        h = half * 6 + hh
                        nc.scalar.copy(
                            vs[:, jt, h * VW:h * VW + 64], ps[:, hh * 64:(hh + 1) * 64]
                        )

        # ---- stage B/C: attention ---------------------------------------
        # i-tiles processed in pairs (256 query cols) so the transposed score
        # matmuls and mask-multiplies run at full fp32r/bf16 rate.
        mpool = ctx.enter_context(tc.tile_pool(name="masks", bufs=2))
        epool = ctx.enter_context(tc.tile_pool(name="ework", bufs=2))
        spool = ctx.enter_context(tc.tile_pool(name="small", bufs=10))
        scps = ctx.enter_context(tc.tile_pool(name="scps", bufs=2, space="PSUM"))
        ups = ctx.enter_context(tc.tile_pool(name="ups", bufs=2, space="PSUM"))
        yps = ctx.enter_context(tc.tile_pool(name="yps", bufs=1, space="PSUM"))

        for itp in range(NIT // 2):
            # transposed-relation onehot masks [j-part, r, jc, i(256)] bf16
            mskT = mpool.tile([128, 6, NJT, 256], BF16, tag="mskT")
            for r in range(1, 7):
                nc.vector.tensor_scalar(
                    mskT[:, r - 1, :, :], relT_sb[:, :, itp * 256:(itp + 1) * 256],
                    float(r), None, ALU.is_equal,
                )

            for h in range(H):
                po = (h % 2) * 64
                mch = h // 2
                qT_p = qTs[po:po + 64, mch, itp * 256:(itp + 1) * 256]

                # transposed scores scT[j, i] = k.q + mask[j], exp -> ET bf16
                ET = epool.tile([128, NJT, 256], BF16, tag="ET")
                for half in range(2):
                    scp = scps.tile([128, 4, 256], F32, tag="sc")
                    for c in range(4):
                        jc = half * 4 + c
                        nc.tensor.matmul(
                            scp[:, c, :],
                            kTs[po:po + 64, mch, jc * 128:(jc + 1) * 128],
                            qT_p, start=True, stop=False,
                        )
                        nc.tensor.matmul(
                            scp[:, c, :],
                            mask8_sb[0:1, jc * 128:(jc + 1) * 128],
                            ones1[:], start=False, stop=True,
                        )
                    nc.scalar.activation(
                        ET[:, half * 4:(half + 1) * 4, :], scp[:], AF.Exp, scale=0.125
                    )

                # per-bin masked copies of ET (pair-wide)
                ErTs = []
                for r in range(1, 7):
                    ErT = epool.tile([128, NJT, 256], BF16, tag=f"ErT{r}")
                    eng = nc.vector if r <= 4 else nc.gpsimd
                    eng.tensor_tensor(ErT[:], ET[:], mskT[:, r - 1, :, :], ALU.mult)
                    ErTs.append(ErT)

                for a in range(2):
                    it = itp * 2 + a
                    isl = slice(a * 128, (a + 1) * 128)
                    qT_h = qTs[po:po + 64, mch, it * 128:(it + 1) * 128]

                    # g = exp(qrel/8) and dg = g_r - g_0
                    qrel = yps.tile([128, 8], F32, tag="y")
                    nc.tensor.matmul(
                        qrel[:], qT_h, rkT_sb[po:po + 64, :], start=True, stop=True,
                    )
                    g = spool.tile([128, 8], F32, tag="g")
                    nc.scalar.activation(g[:, 0:7], qrel[:, 0:7], AF.Exp, scale=0.125)
                    dg = spool.tile([128, 6], F32, tag="dg")
                    nc.vector.tensor_scalar(
                        dg[:], g[:, 1:7], g[:, 0:1], None, ALU.subtract
                    )

                    acc = spool.tile([128, VW], F32, tag="acc")
                    pr = spool.tile([128, 8], F32, tag="pr")
                    u = ups.tile([128, VW], F32, tag="u")
                    for jc in range(NJT):
                        nc.tensor.matmul(
                            u[:], ET[:, jc, isl], vs[:, jc, h * VW:(h + 1) * VW],
                            start=(jc == 0), stop=(jc == NJT - 1),
                        )
                    nc.vector.tensor_scalar(acc[:], u[:], g[:, 0:1], None, ALU.mult)
                    for r in range(1, 7):
                        u = ups.tile([128, VW], F32, tag="u")
                        for jc in range(NJT):
                            nc.tensor.matmul(
                                u[:], ErTs[r - 1][:, jc, isl],
                                vs[:, jc, h * VW:(h + 1) * VW],
                                start=(jc == 0), stop=(jc == NJT - 1),
                            )
                        nc.vector.scalar_tensor_tensor(
                            acc[:], u[:], dg[:, r - 1:r], acc[:],
                            op0=ALU.mult, op1=ALU.add,
                        )
                        nc.vector.tensor_scalar(
                            pr[:, r:r + 1], u[:, 64:65], g[:, r:r + 1], None, ALU.mult
                        )

                    # pr_0 = Z - sum pr_r ; rel-v term; normalize + emit
                    prs = spool.tile([128, 1], F32, tag="prs")
                    nc.vector.tensor_reduce(
                        prs[:], pr[:, 1:7], mybir.AxisListType.X, ALU.add
                    )
                    nc.vector.tensor_scalar(
                        pr[:, 0:1], acc[:, 64:65], prs[:, 0:1], None, ALU.subtract
                    )
                    prT = yps.tile([8, 128], F32, tag="y2")
                    nc.tensor.transpose(prT[0:7, :], pr[:, 0:7], identf[:])
                    prT_sb = spool.tile([8, 128], F32R, tag="prTs")
                    nc.scalar.copy(prT_sb[0:7, :], prT[0:7, :])
                    cxr = yps.tile([128, 64], F32, tag="y2")
                    nc.tensor.matmul(
                        cxr[:], prT_sb[0:7, :], rv_sb[:], start=True, stop=False,
                    )
                    nc.tensor.matmul(
                        cxr[:], identf[:], acc[:, 0:64],
                        start=False, stop=True, skip_group_check=True,
                    )
                    rz = spool.tile([128, 1], F32, tag="rz")
                    nc.vector.reciprocal(rz[:], acc[:, 64:65])
                    nc.scalar.activation(
                        out_sb[:, it, h * 64:(h + 1) * 64], cxr[:], AF.Copy,
                        scale=rz[:],
                    )

            for a in range(2):
                it = itp * 2 + a
                nc.sync.dma_start(out[:, it, :], out_sb[:, it, :])

    nc.compile()
    return nc


_NC_CACHE = []


def _get_nc():
    if not _NC_CACHE:
        _NC_CACHE.append(_build_nc())
    return _NC_CACHE[0]


def _marshal(hidden_states, attention_mask, relation, Wq, bq, Wk, bk, Wv, bv,
             rel_k_emb, rel_v_emb):
    f32 = np.float32
    hidden_states = np.asarray(hidden_states, f32)
    attention_mask = np.asarray(attention_mask, f32)
    relation = np.asarray(relation)
    Wq, Wk, Wv = (np.ascontiguousarray(np.asarray(w, f32)) for w in (Wq, Wk, Wv))
    bq, bk, bv = (np.asarray(x, f32) for x in (bq, bk, bv))

    def wchunk(w):
        return np.ascontiguousarray(w.reshape(NC_CH, 128, HS).transpose(1, 0, 2))

    shared = {
        "wq": wchunk(Wq), "wk": wchunk(Wk), "wv": wchunk(Wv),
        "bqc": np.ascontiguousarray(bq.reshape(NC_CH, 128).T),
        "bkc": np.ascontiguousarray(bk.reshape(NC_CH, 128).T),
        "bvrow": np.ascontiguousarray(bv.reshape(1, HS)),
        "rkT": np.ascontiguousarray(
            np.pad(np.tile(np.asarray(rel_k_emb, f32).T, (2, 1)), ((0, 0), (0, 1)))),
        "rv": np.ascontiguousarray(np.asarray(rel_v_emb, f32)),
    }
    in_maps = []
    for core in range(8):
        b, ih = core // 2, core % 2
        i0 = ih * SH
        hTm = np.ascontiguousarray(hidden_states[b].T)  # [HS, S]
        m = dict(shared)
        m["hT"] = np.ascontiguousarray(hTm.reshape(NC_CH, 128, S).transpose(1, 0, 2))
        m["hTq"] = np.ascontiguousarray(
            hTm[:, i0:i0 + SH].reshape(NC_CH, 128, SH).transpose(1, 0, 2))
        m["relTf"] = np.ascontiguousarray(
            relation[b, i0:i0 + SH].astype(f32).T.reshape(NJT, 128, SH).transpose(1, 0, 2))
        m["mask8"] = np.ascontiguousarray(
            (attention_mask[b, 0, 0] * 8.0).reshape(1, S).astype(f32))
        in_maps.append(m)
    return in_maps


def kernel(hidden_states, attention_mask, relation, Wq, bq, Wk, bk, Wv, bv,
           rel_k_emb, rel_v_emb, _trace=False, _tmpdir=None):
    nc = _get_nc()
    in_maps = _marshal(hidden_states, attention_mask, relation, Wq, bq, Wk, bk,
                       Wv, bv, rel_k_emb, rel_v_emb)
    kw = {}
    if _trace:
        kw = dict(trace=True, tmpdir=_tmpdir)
    res = run_bass_kernel_spmd(nc, in_maps, core_ids=list(range(8)), **kw)
    out = np.zeros((B, S, HS), np.float32)
    for core in range(8):
        b, ih = core // 2, core % 2
        o = res.results[core]["out"]  # [128, NIT, HS]
        out[b, ih * SH:(ih + 1) * SH] = o.transpose(1, 0, 2).reshape(SH, HS)
    if _trace:
        return out, res
    return out
